# revision 1
# baseline (speedup 1.0000x reference)
"""Paged attention with RoPE (decode, B=16, L=1) on 8 trn2 NeuronCores.

Sharding: tensor-parallel over heads. 32 heads / 8 cores = 4 heads per core.
Per core: QKV projection for its heads, RoPE, paged attention over its head
shard of the kv cache, partial output projection; host sums the 8 partials.

Program order is engineered around the per-engine FIFO streams so the DMA
engines (the roofline resource; ~70 MB/core) stay saturated:
  q projection -> paged K gathers + scores (overlap the k/v weight stream)
  -> k projection -> new-token score patches -> v projection
  -> softmax + V phase in two waves of 8 sequences -> output projection.
The reference's reshape_and_cache scatter is never materialized: positions
whose cache slot the new tokens overwrite get their K score patched from a
small q.k_new matrix (16-byte DMAs into the score tile), and their V rows
are zeroed in the gathered tile with the new-token contribution added as a
rank-1 correction matmul after softmax.
"""

import numpy as np

B = 16
HID = 4096
NH = 32
HD = 128
BS = 16
MAXB = 64
NB = 1024
S = MAXB * BS          # 1024 max context
NSLOT = NB * BS        # 16384
N_CORES = 8
HPC = NH // N_CORES    # 4 heads per core
HDPC = HPC * HD        # 512 elements of head-dim per core
KTILES = HID // 128    # 32
NOUT = HID // 512      # 8 output-projection chunks
SCALE = 1.0 / float(np.sqrt(HD))
NEG = -1.0e30


def _plan(position_ids, block_tables, slots, context_lengths, cos_sin_cache):
    """Host-side planning shared by all cores: gather indices, fixups, masks,
    rope tables, per-sequence tile counts."""
    pos = np.clip(position_ids.reshape(B).astype(np.int64), 0,
                  cos_sin_cache.shape[0] - 1)
    bt = block_tables.astype(np.int64)          # [B, MAXB]
    sl = slots.astype(np.int64)                 # [B]
    ctx = context_lengths.astype(np.int64)      # [B]

    # padded gather length: multiple of 128; ctx==0 -> full S, uniform probs
    ctx_eff = np.maximum(ctx, 1)
    pad_len = ((ctx_eff + 127) // 128 * 128).astype(np.int64)
    pad_len = np.where(ctx == 0, S, pad_len)
    nj = (pad_len // 128).astype(np.int64)
    max_nj = int(nj.max())

    p_all = np.arange(S, dtype=np.int64)
    slot_all = bt[:, p_all // BS] * BS + (p_all % BS)      # [B, S]

    # new-token overwrite map: slot -> writing sequence (last writer wins)
    fix = np.full((NSLOT,), -1, dtype=np.int64)
    for bp in range(B):
        fix[sl[bp]] = bp

    fixups = []   # per b: list of (gathered position p, source sequence b')
    for b in range(B):
        n = int(pad_len[b])
        lim = int(min(ctx[b], n)) if ctx[b] > 0 else S
        fb = [(int(p), int(fix[slot_all[b, p]]))
              for p in range(lim) if fix[slot_all[b, p]] >= 0]
        fixups.append(fb)

    # int16 gather index tiles, wrapped mod 16 partitions, replicated x8
    idx_tiles = np.zeros((B, 128, S // 16), dtype=np.int16)
    for b in range(B):
        n = int(pad_len[b])
        idx = slot_all[b, :n].copy()
        lim = int(min(ctx[b], n)) if ctx[b] > 0 else S
        idx[lim:] = idx[0]                     # pad with a valid row
        wrapped = idx.reshape(n // 16, 16).T   # [16, n/16]
        idx_tiles[b, :, : n // 16] = np.tile(wrapped, (8, 1))

    # additive mask [64 rows = b*HPC+h, max context]
    mask = np.zeros((64, S), dtype=np.float32)
    for b in range(B):
        if ctx[b] > 0:
            mask[b * HPC:(b + 1) * HPC, int(ctx[b]):] = NEG
        else:
            mask[b * HPC:(b + 1) * HPC, :] = NEG   # uniform over full S
    mask = np.ascontiguousarray(mask[:, :max_nj * 128])

    # rope tables, per-head replicated, sin sign baked ( [-sin | +sin] )
    cs = cos_sin_cache[pos]                     # [B, 128]
    cos_h, sin_h = cs[:, :64], cs[:, 64:]
    cos_full = np.concatenate([cos_h, cos_h], axis=1)
    sin_sign = np.concatenate([-sin_h, sin_h], axis=1)
    cos_rep = np.ascontiguousarray(np.tile(cos_full, (1, HPC)), dtype=np.float32)
    sin_rep = np.ascontiguousarray(np.tile(sin_sign, (1, HPC)), dtype=np.float32)

    return {
        'nj': [int(x) for x in nj], 'max_nj': max_nj, 'fixups': fixups,
        'idx_tiles': idx_tiles, 'mask': mask,
        'cos_rep': cos_rep, 'sin_rep': sin_rep,
    }


def _build_bass(plan):
    """Build the per-core bass program (identical program for every core;
    only the input data differs)."""
    import concourse.tile as tile
    from concourse import bacc, mybir
    from concourse.masks import make_identity
    from contextlib import ExitStack

    fp32 = mybir.dt.float32
    f32r = mybir.dt.float32r
    i16 = mybir.dt.int16
    AX = mybir.AxisListType
    ALU = mybir.AluOpType
    ACTF = mybir.ActivationFunctionType

    nj = plan['nj']
    max_nj = plan['max_nj']
    fixups = plan['fixups']
    SW = max_nj * 128          # score width
    n_fix = sum(len(fb) for fb in fixups)

    nc = bacc.Bacc("TRN2", target_bir_lowering=False, debug=False,
                   num_devices=N_CORES)

    hiddenT = nc.dram_tensor("hiddenT", [HID, B], fp32, kind="ExternalInput")
    wqkvT = nc.dram_tensor("wqkvT", [HID, 3 * HDPC], fp32, kind="ExternalInput")
    woT = nc.dram_tensor("woT", [HDPC, HID], fp32, kind="ExternalInput")
    ksrc = nc.dram_tensor("ksrc", [NSLOT, HDPC], fp32, kind="ExternalInput")
    vsrc = nc.dram_tensor("vsrc", [NSLOT, HDPC], fp32, kind="ExternalInput")
    idxs = nc.dram_tensor("idxs", [B, 128, S // 16], i16, kind="ExternalInput")
    maskd = nc.dram_tensor("maskd", [64, SW], fp32, kind="ExternalInput")
    cosd = nc.dram_tensor("cosd", [B, HDPC], fp32, kind="ExternalInput")
    sind = nc.dram_tensor("sind", [B, HDPC], fp32, kind="ExternalInput")
    y = nc.dram_tensor("y", [B, HID], fp32, kind="ExternalOutput")

    with tile.TileContext(nc) as tc, ExitStack() as ctx:
        const_p = ctx.enter_context(tc.tile_pool(name="const", bufs=1))
        w_p = ctx.enter_context(tc.tile_pool(name="w", bufs=6))
        kv_p = ctx.enter_context(tc.tile_pool(name="kv", bufs=3))
        vg_p = ctx.enter_context(tc.tile_pool(name="vgp", bufs=3))
        sb_p = ctx.enter_context(tc.tile_pool(name="sb", bufs=1))
        tmp_p = ctx.enter_context(tc.tile_pool(name="tmp", bufs=2))
        psacc = ctx.enter_context(tc.tile_pool(name="psacc", bufs=2, space="PSUM"))
        pssm = ctx.enter_context(tc.tile_pool(name="pssm", bufs=1, space="PSUM"))
        psat = ctx.enter_context(tc.tile_pool(name="psat", bufs=1, space="PSUM"))
        psyt = ctx.enter_context(tc.tile_pool(name="psyt", bufs=1, space="PSUM"))
        psqr = ctx.enter_context(tc.tile_pool(name="psqr", bufs=1, space="PSUM"))

        # ---------------- constants ----------------
        ident = const_p.tile([128, 128], fp32)
        make_identity(nc, ident[:])
        zrow = const_p.tile([1, HDPC], fp32)
        nc.vector.memset(zrow[:], 0)
        ht_sb = const_p.tile([128, KTILES * B], fp32)
        nc.sync.dma_start(ht_sb[:].rearrange("p (t b) -> p t b", b=B),
                            hiddenT.ap().rearrange("(t p) b -> p t b", p=128))
        cos_sb = const_p.tile([B, HDPC], fp32)
        nc.sync.dma_start(cos_sb[:], cosd.ap())
        sin_sb = const_p.tile([B, HDPC], fp32)
        nc.sync.dma_start(sin_sb[:], sind.ap())
        mask_sb = const_p.tile([32, 2 * SW], fp32)
        nc.sync.dma_start(mask_sb[:].rearrange("p (g w) -> p g w", g=2),
                          maskd.ap().rearrange("(g p) w -> p g w", g=2))
        idx_sb = const_p.tile([128, B * (S // 16)], i16)
        nc.sync.dma_start(idx_sb[:].rearrange("p (b c) -> p b c", b=B),
                          idxs.ap().rearrange("b p c -> p b c"))

        def rope(dst, src):
            src3 = src.rearrange("b (h two d) -> b h two d", two=2, d=64)
            rot = tmp_p.tile([B, HDPC], fp32, name="rot", tag="rot", bufs=1)
            rot3 = rot[:].rearrange("b (h two d) -> b h two d", two=2, d=64)
            nc.vector.tensor_copy(rot3[:, :, 0, :], src3[:, :, 1, :])
            nc.vector.tensor_copy(rot3[:, :, 1, :], src3[:, :, 0, :])
            nc.vector.tensor_mul(rot[:], rot[:], sin_sb[:])
            cp = tmp_p.tile([B, HDPC], fp32, name="cp", tag="cp", bufs=1)
            nc.vector.tensor_mul(cp[:], src, cos_sb[:])
            nc.vector.tensor_add(dst[:], cp[:], rot[:])

        q_sb = sb_p.tile([B, HDPC], fp32)
        k_sb = sb_p.tile([B, HDPC], fp32)
        v_sb = sb_p.tile([B, HDPC], fp32)

        def wpass(col0, out_ps):
            for kt in range(KTILES):
                wt = w_p.tile([128, HPC * 512], fp32, name="wt", tag="w",
                              padded_shape=[128, HPC * 512])
                nc.sync.dma_start(wt[:, :HDPC],
                                    wqkvT.ap()[kt * 128:(kt + 1) * 128,
                                               col0:col0 + HDPC])
                nc.tensor.matmul(out_ps[:],
                                 ht_sb[:, kt * B:(kt + 1) * B],
                                 wt[:, :HDPC],
                                 start=(kt == 0), stop=(kt == KTILES - 1))

        # ---------------- q projection (first, to unblock scores) ----------
        q_ps = pssm.tile([B, HDPC], fp32, name="q_ps", tag="sm")
        wpass(0, q_ps)
        rope(q_sb, q_ps[:])

        # ---------------- K gather + scores ----------------
        swide = sb_p.tile([128, max_nj * 64], fp32)
        nc.gpsimd.memset(swide[:], 0)

        def qrep_bcast(b):
            # qrep = row b of q_sb on all partitions: eye-column broadcast
            # matmul (lhsT[k, m] = ident[k, b] for every m).
            qr_ps = psqr.tile([128, HDPC], fp32, name="qr_ps", tag="qr")
            nc.tensor.matmul(qr_ps[:],
                             ident[:B, b:b + 1].to_broadcast([B, 128])
                             ,
                             q_sb[:], start=True, stop=True)
            qrep = tmp_p.tile([128, HDPC], fp32, name="qrep", tag="qrep",
                              bufs=4)
            nc.scalar.copy(qrep[:], qr_ps[:])
            return qrep

        def score_tile(b, j, src, qrep):
            prod = tmp_p.tile([128, HDPC], fp32, name="prod", tag="prod")
            nc.vector.tensor_mul(prod[:], src, qrep[:])
            nc.vector.tensor_reduce(
                out=swide[:, j * 64 + b * HPC: j * 64 + (b + 1) * HPC],
                in_=prod[:].rearrange("p (h d) -> p h d", d=HD),
                axis=AX.X, op=ALU.add)

        for b in range(B):
            n = nj[b] * 128
            kg = kv_p.tile([128, max_nj * HDPC], fp32, name="kg", tag="kvg")
            nc.gpsimd.dma_gather(
                out_ap=kg[:].rearrange("p (j e) -> p j e", e=HDPC)[:, :nj[b], :],
                in_ap=ksrc.ap(),
                idxs_ap=idx_sb[:, b * (S // 16): b * (S // 16) + n // 16],
                num_idxs=n, num_idxs_reg=n, elem_size=HDPC)
            qrep = qrep_bcast(b)
            for j in range(nj[b]):
                score_tile(b, j, kg[:, j * HDPC:(j + 1) * HDPC], qrep)

        # ---------------- k projection + new-token score patches -----------
        k_ps = pssm.tile([B, HDPC], fp32, name="k_ps", tag="sm")
        wpass(HDPC, k_ps)
        rope(k_sb, k_ps[:])

        # F[b', 4b+h] = q_{b,h} . k_new_{b',h} via four [16,16] matmuls on
        # transposed q/k slices; patch affected swide cells with 16B DMAs.
        if n_fix:
            qkT = sb_p.tile([128, 2 * HPC * B], fp32)   # [d, (qk, h, b)]
            for s_i, src in ((0, q_sb), (1, k_sb)):
                for h in range(HPC):
                    tp = pssm.tile([128, B], fp32, name="tp", tag="sm")
                    nc.tensor.transpose(tp[:], src[:, h * HD:(h + 1) * HD],
                                        ident[:B, :B])
                    nc.scalar.copy(
                        qkT[:, (s_i * HPC + h) * B:(s_i * HPC + h + 1) * B],
                        tp[:])
            F_ps = pssm.tile([B, HPC * B], fp32, name="F_ps", tag="sm")
            for h in range(HPC):
                nc.tensor.matmul(F_ps[:, h:HPC * B:HPC],
                                 qkT[:, (HPC + h) * B:(HPC + h + 1) * B],
                                 qkT[:, h * B:(h + 1) * B],
                                 start=True, stop=True)
            F_sb = sb_p.tile([B, HPC * B], fp32)
            nc.scalar.copy(F_sb[:], F_ps[:])
            for b in range(B):
                for (p, bp) in fixups[b]:
                    nc.sync.dma_start(
                        swide[p % 128:p % 128 + 1,
                              (p // 128) * 64 + b * HPC:
                              (p // 128) * 64 + (b + 1) * HPC],
                        F_sb[bp:bp + 1, b * HPC:(b + 1) * HPC])

        # ---------------- v projection ----------------
        v_ps = pssm.tile([B, HDPC], fp32, name="v_ps", tag="sm")
        wpass(2 * HDPC, v_ps)
        nc.vector.tensor_copy(v_sb[:], v_ps[:])

        # wo prefetch (slot rotation lets these stream in during the V phase)
        wo_tiles = []
        for i in range(NOUT):
            if i < NOUT - 2:
                wo = w_p.tile([128, HPC * 512], fp32, name="wo", tag="w")
            else:
                wo = kv_p.tile([128, HPC * 512], fp32, name="wo", tag="kvg")
            nc.sync.dma_start(
                wo[:].rearrange("p (t c) -> p t c", t=HPC),
                woT.ap()[:, i * 512:(i + 1) * 512]
                .rearrange("(t p) c -> p t c", p=128))
            wo_tiles.append(wo)

        # -------- softmax + V phase, two waves of 8 sequences each ---------
        pT = sb_p.tile([128, max_nj * 64], fp32)
        p_waves = []
        atT_ps = psat.tile([128, 64], fp32)      # cols h*16+b
        # p-scalars for the rank-1 new-token corrections, cell [b', b*4+h];
        # multiple fixups landing in the same column sum in the matmul, but a
        # duplicated (b, b') pair needs an overflow column of its own.
        flat_fixups = [(b, p, bp) for b in range(B) for (p, bp) in fixups[b]]
        seen, base_fix, extra_fix = set(), [], []
        for (b, p, bp) in flat_fixups:
            if (b, bp) in seen:
                extra_fix.append((b, p, bp))
            else:
                seen.add((b, bp))
                base_fix.append((b, p, bp))
        n_extra = len(extra_fix)
        psc = sb_p.tile([B, 4 * B + 4 * max(1, n_extra)], fp32)
        nc.vector.memset(psc[:], 0)

        for g in range(2):
            r0, r1 = 32 * g, 32 * (g + 1)        # bh rows of this wave
            sc_ps = psacc.tile([32, SW], fp32, name="sc_ps", tag="acc")
            for j in range(max_nj):
                nc.tensor.transpose(
                    sc_ps[:, j * 128:(j + 1) * 128],
                    swide[:, j * 64 + r0: j * 64 + r1], ident[:])
            sc = sb_p.tile([32, SW], fp32, name="sc", tag="sc", bufs=1)
            nc.vector.scalar_tensor_tensor(
                out=sc[:], in0=sc_ps[:], scalar=SCALE,
                in1=mask_sb[:, g * SW:(g + 1) * SW], op0=ALU.mult, op1=ALU.add)
            negmax = sb_p.tile([32, 1], fp32, name="negmax", tag="nm", bufs=2)
            nc.vector.tensor_reduce(out=negmax[:], in_=sc[:],
                                    axis=AX.X, op=ALU.max, negate=True)
            p_sb = sb_p.tile([32, SW], fp32, name="p_sb", tag=f"pw{g}")
            sums = sb_p.tile([32, 1], fp32, name="sums", tag="sums", bufs=2)
            nc.scalar.activation(out=p_sb[:], in_=sc[:],
                                 func=ACTF.Exp, bias=negmax[:],
                                 scale=1.0, accum_out=sums[:])
            rsum = sb_p.tile([32, 1], fp32, name="rsum", tag="rs", bufs=2)
            nc.vector.reciprocal(rsum[:], sums[:])
            nc.vector.tensor_scalar_mul(p_sb[:], p_sb[:], rsum[:])
            p_waves.append(p_sb)
            for j in range(max_nj):
                pt_ps = pssm.tile([128, 32], fp32, name="pt_ps", tag="sm")
                nc.tensor.transpose(pt_ps[:], p_sb[:, j * 128:(j + 1) * 128],
                                    ident[:32, :32])
                nc.scalar.copy(pT[:, j * 64 + r0: j * 64 + r1], pt_ps[:])

            # p-scalars for this wave's rank-1 corrections
            for (b, p, bp) in base_fix:
                if 8 * g <= b < 8 * (g + 1):
                    nc.sync.dma_start(
                        psc[bp:bp + 1, 4 * b:4 * b + 4],
                        p_sb[(b - 8 * g) * HPC:(b - 8 * g + 1) * HPC, p:p + 1])
            for f, (b, p, bp) in enumerate(extra_fix):
                if 8 * g <= b < 8 * (g + 1):
                    nc.sync.dma_start(
                        psc[bp:bp + 1, 4 * B + 4 * f:4 * B + 4 * f + 4],
                        p_sb[(b - 8 * g) * HPC:(b - 8 * g + 1) * HPC, p:p + 1])

            for b in range(8 * g, 8 * g + 8):
                vg = vg_p.tile([128, max_nj * HDPC], fp32, name="vg", tag="vgt")
                n = nj[b] * 128
                nc.gpsimd.dma_gather(
                    out_ap=vg[:].rearrange("p (j e) -> p j e",
                                           e=HDPC)[:, :nj[b], :],
                    in_ap=vsrc.ap(),
                    idxs_ap=idx_sb[:, b * (S // 16): b * (S // 16) + n // 16],
                    num_idxs=n, num_idxs_reg=n, elem_size=HDPC)
                # zero the overwritten rows: their stale contribution must
                # vanish; the new-token term is added as a rank-1 correction.
                for (p, bp) in fixups[b]:
                    nc.sync.dma_start(
                        vg[p % 128:p % 128 + 1,
                           (p // 128) * HDPC:(p // 128 + 1) * HDPC], zrow[:])
                for h in range(HPC):
                    col = h * B + b
                    for j in range(nj[b]):
                        nc.tensor.matmul(
                            atT_ps[:, col:col + 1],
                            vg[:, j * HDPC + h * HD: j * HDPC + (h + 1) * HD]
                            ,
                            pT[:, j * 64 + b * HPC + h:
                               j * 64 + b * HPC + h + 1],
                            start=(j == 0), stop=(j == nj[b] - 1))

        # rank-1 corrections: attnT[:, h*16+b] += p[p*] * v_new[b'], computed
        # in a separate PSUM tile and folded in with the PSUM->SBUF move.
        attnT = sb_p.tile([128, 64], fp32)
        if flat_fixups:
            C_ps = pssm.tile([128, 64], fp32, name="C_ps", tag="sm")
            for h in range(HPC):
                nc.tensor.matmul(C_ps[:, h * B:(h + 1) * B],
                                 v_sb[:, h * HD:(h + 1) * HD],
                                 psc[:, h:4 * B:HPC],
                                 start=True, stop=True)
                for f, (b, p, bp) in enumerate(extra_fix):
                    nc.tensor.matmul(
                        C_ps[:, h * B + b: h * B + b + 1],
                        v_sb[:, h * HD:(h + 1) * HD],
                        psc[:, 4 * B + 4 * f + h: 4 * B + 4 * f + h + 1],
                        start=False, stop=True, skip_group_check=True)
            C_sb = sb_p.tile([128, 64], fp32)
            nc.scalar.copy(C_sb[:], C_ps[:])
            nc.vector.tensor_add(attnT[:], atT_ps[:], C_sb[:])
        else:
            nc.scalar.copy(attnT[:], atT_ps[:])

        # ------------- output projection (4 chunks packed per PE pass) ------
        for r in range(NOUT // 4):
            yt = psyt.tile([128, 512], fp32, name="yt", tag="yt")
            for c in range(4):
                i = r * 4 + c
                for h in range(HPC):
                    nc.tensor.matmul(yt[32 * c:32 * c + B, :],
                                     attnT[:, h * B:(h + 1) * B],
                                     wo_tiles[i][:, h * 512:(h + 1) * 512]
                                     ,
                                     start=(h == 0), stop=(h == HPC - 1),
                                     tile_position=(0, 32 * c))
            yst = tmp_p.tile([128, 512], fp32, name="yst", tag="yst")
            for c in range(4):
                nc.scalar.copy(yst[32 * c:32 * c + B, :], yt[32 * c:32 * c + B, :])
            for c in range(4):
                i = r * 4 + c
                nc.sync.dma_start(y.ap()[:, i * 512:(i + 1) * 512],
                                  yst[32 * c:32 * c + B, :])

    nc.compile()
    return nc


def _make_in_maps(hidden_states, qkv_w, out_w, key_cache, value_cache, plan):
    hid = hidden_states.reshape(B, HID).astype(np.float32)
    hiddenT = np.ascontiguousarray(hid.T)

    wq, wk, wv = qkv_w[:HID], qkv_w[HID:2 * HID], qkv_w[2 * HID:]
    kc = key_cache.reshape(NSLOT, NH, HD)
    vc = value_cache.reshape(NSLOT, NH, HD)

    in_maps = []
    for c in range(N_CORES):
        h0 = c * HPC
        r0, r1 = h0 * HD, (h0 + HPC) * HD
        wqkvT = np.ascontiguousarray(
            np.concatenate([wq[r0:r1], wk[r0:r1], wv[r0:r1]], axis=0).T)
        woT = np.ascontiguousarray(out_w[:, r0:r1].T)
        ks = np.ascontiguousarray(kc[:, h0:h0 + HPC, :].reshape(NSLOT, HDPC))
        vs = np.ascontiguousarray(vc[:, h0:h0 + HPC, :].reshape(NSLOT, HDPC))
        in_maps.append({
            "hiddenT": hiddenT, "wqkvT": wqkvT, "woT": woT,
            "ksrc": ks, "vsrc": vs,
            "idxs": plan['idx_tiles'], "maskd": plan['mask'],
            "cosd": plan['cos_rep'], "sind": plan['sin_rep'],
        })
    return in_maps


def kernel(hidden_states, qkv_w, out_w, cos_sin_cache, key_cache, value_cache,
           position_ids, block_tables, slots, context_lengths):
    from concourse.bass_utils import run_bass_kernel_spmd

    hidden_states = np.asarray(hidden_states, dtype=np.float32)
    qkv_w = np.asarray(qkv_w, dtype=np.float32)
    out_w = np.asarray(out_w, dtype=np.float32)
    cos_sin_cache = np.asarray(cos_sin_cache, dtype=np.float32)
    key_cache = np.asarray(key_cache, dtype=np.float32)
    value_cache = np.asarray(value_cache, dtype=np.float32)

    plan = _plan(np.asarray(position_ids), np.asarray(block_tables),
                 np.asarray(slots), np.asarray(context_lengths), cos_sin_cache)
    nc = _build_bass(plan)
    in_maps = _make_in_maps(hidden_states, qkv_w, out_w, key_cache,
                            value_cache, plan)

    res = run_bass_kernel_spmd(nc, in_maps, core_ids=list(range(N_CORES)))
    out = np.zeros((B, HID), dtype=np.float32)
    for c in range(N_CORES):
        out += res.results[c]["y"]
    return out.reshape(B, 1, HID)



# revision 52
# speedup vs baseline: 2.7141x; 2.7141x over previous
"""Paged attention with RoPE (decode, B=16, L=1) on 8 trn2 NeuronCores.

Sharding: tensor-parallel over heads. 32 heads / 8 cores = 4 heads per core.
Per core: QKV projection for its heads, RoPE, paged attention over its head
shard of the kv cache, partial output projection; host sums the 8 partials.

The kernel is engineered around the serialized DMA stream (the roofline
resource): all bulk traffic (weights, KV gathers) moves in bf16, halving the
byte count vs fp32. K is gathered with dma_gather(transpose=True) so the
head dim lands on partitions and attention scores become tensor-engine
matmuls (N=1 per 128-token chunk) instead of vector-engine mul+reduce.
Softmax runs in a single 64-row pass ([b*4+h, S]).

The reference's reshape_and_cache scatter is never materialized: gather
indices of cache slots that the new tokens overwrite are redirected to a
zero row appended to the K/V sources. The new-token K scores are patched
into the score tile with partition-aligned vector copies from an on-chip
F = q . k_new matrix, and the new-token V contribution is added as
PSUM-accumulated rank-1 matmuls using p* = exp(F - max) / sum recomputed
from the same softmax statistics.
"""

import numpy as np

B = 16
HID = 4096
NH = 32
HD = 128
BS = 16
MAXB = 64
NB = 1024
S = MAXB * BS          # 1024 max context
NSLOT = NB * BS        # 16384
ZROW = NSLOT           # index of the appended all-zero cache row
N_CORES = 8
HPC = NH // N_CORES    # 4 heads per core
HDPC = HPC * HD        # 512 elements of head-dim per core
KTILES = HID // 128    # 32
NOUT = HID // 512      # 8 output-projection chunks
SCALE = 1.0 / float(np.sqrt(HD))
NEG = -1.0e30


def _plan(position_ids, block_tables, slots, context_lengths, cos_sin_cache):
    """Host-side planning shared by all cores: gather indices, fixups, masks,
    rope tables, per-sequence tile counts."""
    pos = np.clip(position_ids.reshape(B).astype(np.int64), 0,
                  cos_sin_cache.shape[0] - 1)
    bt = block_tables.astype(np.int64)          # [B, MAXB]
    sl = slots.astype(np.int64)                 # [B]
    ctx = np.maximum(context_lengths.astype(np.int64), 1)  # reference: ctx>=1

    # K gathers (transpose mode) need multiples of 128; V gathers of 16.
    n128 = ((ctx + 127) // 128 * 128).astype(np.int64)
    n16 = ((ctx + 15) // 16 * 16).astype(np.int64)
    nj = (n128 // 128).astype(np.int64)
    njv = ((n16 + 127) // 128).astype(np.int64)
    max_nj = int(nj.max())

    p_all = np.arange(S, dtype=np.int64)
    slot_all = bt[:, p_all // BS] * BS + (p_all % BS)      # [B, S]

    # new-token overwrite map: slot -> writing sequence (slots are unique)
    fix = np.full((NSLOT,), -1, dtype=np.int64)
    for bp in range(B):
        fix[sl[bp]] = bp

    fixups = []   # per b: list of (gathered position p, source sequence b')
    for b in range(B):
        fixups.append([(int(p), int(fix[slot_all[b, p]]))
                       for p in range(int(ctx[b]))
                       if fix[slot_all[b, p]] >= 0])

    # int16 gather index tiles, wrapped mod 16 partitions, replicated x8.
    # Positions >= ctx (padding) and overwritten slots read the zero row.
    idx_tiles = np.zeros((B, 128, S // 16), dtype=np.int16)
    for b in range(B):
        n = int(n128[b])
        idx = slot_all[b, :n].copy()
        idx[int(ctx[b]):] = ZROW
        for (p, _) in fixups[b]:
            idx[p] = ZROW
        wrapped = idx.reshape(n // 16, 16).T   # [16, n/16]
        idx_tiles[b, :, : n // 16] = np.tile(wrapped, (8, 1))

    # additive mask [64 rows = b*HPC+h, max context]
    mask = np.zeros((64, S), dtype=np.float32)
    for b in range(B):
        mask[b * HPC:(b + 1) * HPC, int(ctx[b]):] = NEG
    mask = np.ascontiguousarray(mask[:, :max_nj * 128])

    # 1.0 where (b, b') is a real fixup pair, else 0 — gates the rank-1
    # new-token V correction (rows b*4+h, cols b') and, transposed, the
    # score-patch matmuls (rows b', cols b*4+h).
    fixm = np.zeros((64, B), dtype=np.float32)
    for b in range(B):
        for (_, bp) in fixups[b]:
            fixm[b * HPC:(b + 1) * HPC, bp] = 1.0

    # patches are applied as PE matmuls gated by fixm^T; that only stays
    # collision-free when each writer b' patches a single sequence (true
    # whenever block tables are disjoint, which the reference guarantees —
    # every fixup is then (b, ctx_b-1, b)). Cross fixups fall back to DMA.
    writers = [bp for fb in fixups for (_, bp) in fb]
    diag_ok = len(writers) == len(set(writers))

    # rope tables, per-head replicated, sin sign baked ( [-sin | +sin] );
    # the q tables additionally fold in the 1/sqrt(HD) score scale.
    cs = cos_sin_cache[pos]                     # [B, 128]
    cos_h, sin_h = cs[:, :64], cs[:, 64:]
    cos_full = np.concatenate([cos_h, cos_h], axis=1)
    sin_sign = np.concatenate([-sin_h, sin_h], axis=1)
    cos_rep = np.ascontiguousarray(np.tile(cos_full, (1, HPC)), dtype=np.float32)
    sin_rep = np.ascontiguousarray(np.tile(sin_sign, (1, HPC)), dtype=np.float32)

    return {
        'nj': [int(x) for x in nj], 'njv': [int(x) for x in njv],
        'n16': [int(x) for x in n16], 'max_nj': max_nj, 'fixups': fixups,
        'idx_tiles': idx_tiles, 'mask': mask, 'fixm': fixm,
        'fixmT': np.ascontiguousarray(fixm.T), 'diag_ok': diag_ok,
        'cos_rep': cos_rep, 'sin_rep': sin_rep,
        'qcos': cos_rep * SCALE, 'qsin': sin_rep * SCALE,
    }


def _build_bass(plan, debug=False, stage=99):
    """Build the per-core bass program (identical program for every core;
    only the input data differs)."""
    import concourse.tile as tile
    from concourse import bacc, mybir
    from concourse.masks import make_identity
    from contextlib import ExitStack

    fp32 = mybir.dt.float32
    bf16 = mybir.dt.bfloat16
    i16 = mybir.dt.int16
    AX = mybir.AxisListType
    ALU = mybir.AluOpType
    ACTF = mybir.ActivationFunctionType

    nj = plan['nj']
    njv = plan['njv']
    n16 = plan['n16']
    max_nj = plan['max_nj']
    fixups = plan['fixups']
    SW = max_nj * 128          # score width

    nc = bacc.Bacc("TRN2", target_bir_lowering=False, debug=False,
                   num_devices=N_CORES)

    hiddenT = nc.dram_tensor("hiddenT", [HID, B], bf16, kind="ExternalInput")
    wqkvT = nc.dram_tensor("wqkvT", [HID, 3 * HDPC], bf16, kind="ExternalInput")
    woT = nc.dram_tensor("woT", [HDPC, HID], bf16, kind="ExternalInput")
    ksrc = nc.dram_tensor("ksrc", [NSLOT + 1, HDPC], bf16, kind="ExternalInput")
    vsrc = nc.dram_tensor("vsrc", [NSLOT + 1, HDPC], bf16, kind="ExternalInput")
    idxs = nc.dram_tensor("idxs", [B, 128, S // 16], i16, kind="ExternalInput")
    maskd = nc.dram_tensor("maskd", [64, SW], fp32, kind="ExternalInput")
    fixmd = nc.dram_tensor("fixmd", [64, B], fp32, kind="ExternalInput")
    fixmTd = nc.dram_tensor("fixmTd", [B, 64], fp32, kind="ExternalInput")
    qcosd = nc.dram_tensor("qcosd", [B, HDPC], fp32, kind="ExternalInput")
    qsind = nc.dram_tensor("qsind", [B, HDPC], fp32, kind="ExternalInput")
    kcosd = nc.dram_tensor("kcosd", [B, HDPC], fp32, kind="ExternalInput")
    ksind = nc.dram_tensor("ksind", [B, HDPC], fp32, kind="ExternalInput")
    y = nc.dram_tensor("y", [B, HID], fp32, kind="ExternalOutput")
    dump_list = []

    def dump(name, ap):
        if not debug:
            return
        d = nc.dram_tensor(f"dbg_{name}", list(ap.shape), ap.dtype,
                           kind="ExternalOutput")
        nc.sync.dma_start(d.ap(), ap)
        dump_list.append(name)

    def _body():
      with tile.TileContext(nc) as tc, ExitStack() as ctx:
        const_p = ctx.enter_context(tc.tile_pool(name="const", bufs=1))
        w_p = ctx.enter_context(tc.tile_pool(name="w", bufs=2))
        wo_p = ctx.enter_context(tc.tile_pool(name="wop", bufs=1))
        kv_p = ctx.enter_context(tc.tile_pool(name="kv", bufs=3))
        vg_p = ctx.enter_context(tc.tile_pool(name="vgp", bufs=1))
        sb_p = ctx.enter_context(tc.tile_pool(name="sb", bufs=1))
        tmp_p = ctx.enter_context(tc.tile_pool(name="tmp", bufs=2))
        pssc = ctx.enter_context(tc.tile_pool(name="pssc", bufs=1, space="PSUM"))
        pssm = ctx.enter_context(tc.tile_pool(name="pssm", bufs=1, space="PSUM"))
        pstp = ctx.enter_context(tc.tile_pool(name="pstp", bufs=2, space="PSUM"))
        psat = ctx.enter_context(tc.tile_pool(name="psat", bufs=1, space="PSUM"))
        psyt = ctx.enter_context(tc.tile_pool(name="psyt", bufs=1, space="PSUM"))

        # ---------------- constants ----------------
        identf = const_p.tile([128, 128], fp32)
        make_identity(nc, identf[:])
        identb = const_p.tile([128, 128], bf16)
        nc.vector.tensor_copy(identb[:], identf[:])
        ht_sb = const_p.tile([128, KTILES * B], bf16)
        nc.sync.dma_start(ht_sb[:].rearrange("p (t b) -> p t b", b=B),
                          hiddenT.ap().rearrange("(t p) b -> p t b", p=128))
        qcos_sb = const_p.tile([B, HDPC], fp32)
        nc.sync.dma_start(qcos_sb[:], qcosd.ap())
        qsin_sb = const_p.tile([B, HDPC], fp32)
        nc.sync.dma_start(qsin_sb[:], qsind.ap())
        kcos_sb = const_p.tile([B, HDPC], fp32)
        nc.sync.dma_start(kcos_sb[:], kcosd.ap())
        ksin_sb = const_p.tile([B, HDPC], fp32)
        nc.sync.dma_start(ksin_sb[:], ksind.ap())
        mask_sb = const_p.tile([64, SW], fp32)
        nc.sync.dma_start(mask_sb[:], maskd.ap())
        fixm_sb = const_p.tile([64, B], fp32)
        nc.sync.dma_start(fixm_sb[:], fixmd.ap())
        fixmT_sb = const_p.tile([B, 64], fp32)
        nc.sync.dma_start(fixmT_sb[:], fixmTd.ap())
        idx_sb = const_p.tile([128, B * (S // 16)], i16)
        nc.sync.dma_start(idx_sb[:].rearrange("p (b c) -> p b c", b=B),
                          idxs.ap().rearrange("b p c -> p b c"))

        def rope(dst, src, cos_sb, sin_sb):
            src3 = src.rearrange("b (h two d) -> b h two d", two=2, d=64)
            rot = tmp_p.tile([B, HDPC], fp32, name="rot", tag="rot", bufs=1)
            rot3 = rot[:].rearrange("b (h two d) -> b h two d", two=2, d=64)
            nc.vector.tensor_copy(rot3[:, :, 0, :], src3[:, :, 1, :])
            nc.vector.tensor_copy(rot3[:, :, 1, :], src3[:, :, 0, :])
            nc.vector.tensor_mul(rot[:], rot[:], sin_sb[:])
            cp = tmp_p.tile([B, HDPC], fp32, name="cp", tag="cp", bufs=1)
            nc.vector.tensor_mul(cp[:], src, cos_sb[:])
            nc.vector.tensor_add(dst[:], cp[:], rot[:])

        q_sb = sb_p.tile([B, HDPC], bf16)
        k_sb = sb_p.tile([B, HDPC], bf16)
        v_sb = sb_p.tile([B, HDPC], bf16)

        WCHUNK = 8  # weight k-tiles per DMA

        def wpass(col0, out_ps):
            for wc in range(KTILES // WCHUNK):
                wt = w_p.tile([128, WCHUNK * HDPC], bf16, name="wt", tag="w")
                nc.sync.dma_start(
                    wt[:].rearrange("p (t c) -> p t c", t=WCHUNK),
                    wqkvT.ap()[wc * WCHUNK * 128:(wc + 1) * WCHUNK * 128,
                               col0:col0 + HDPC]
                    .rearrange("(t p) c -> p t c", p=128))
                for t in range(WCHUNK):
                    kt = wc * WCHUNK + t
                    nc.tensor.matmul(out_ps[:],
                                     ht_sb[:, kt * B:(kt + 1) * B],
                                     wt[:, t * HDPC:(t + 1) * HDPC],
                                     start=(kt == 0), stop=(kt == KTILES - 1))

        # transpose a [B, HDPC] sbuf tile into dstT [128, 64] (cols b*4+h)
        def head_transpose(dstT, src_sb):
            for h in range(HPC):
                tp = pstp.tile([128, B], bf16, name="tp", tag="tp")
                nc.tensor.transpose(tp[:], src_sb[:, h * HD:(h + 1) * HD],
                                    identb[:B, :B])
                nc.scalar.copy(dstT[:, h:4 * B:HPC], tp[:])

        # ---------------- q projection (first, to unblock scores) ----------
        q_ps = pssm.tile([B, HDPC], fp32, name="q_ps", tag="sm")
        wpass(0, q_ps)
        rope(q_sb, q_ps[:], qcos_sb, qsin_sb)   # scale baked into q tables
        qT = sb_p.tile([128, 4 * B], bf16)      # [d, b*4+h]
        head_transpose(qT, q_sb)
        dump('q_sb', q_sb[:])
        dump('qT', qT[:])
        if stage <= 1:
            qf = sb_p.tile([B, HDPC], fp32, name="qf", tag="qf")
            nc.vector.tensor_copy(qf[:], q_sb[:])
            nc.sync.dma_start(y.ap()[:, :HDPC], qf[:])
            return

        # ---------------- K gather (transposed) + scores --------------------
        # kgT[b]: [128 d, 4 h, nj*128 s] via transpose-mode gather; score
        # s[token, b*4+h] = kgT_h^T . qT col — N=1 matmuls into PSUM swide.
        swide_ps = pssc.tile([128, max_nj * 64], fp32, name="swide", tag="sw",
                             padded_shape=[128, 512])
        nc.vector.memset(swide_ps[:], 0)
        for b in range(B):
            n = nj[b] * 128
            kgs = []   # (tile, chunk token count): transpose gathers cap
            for c0 in range(0, n, 512):   # below 1024 idxs; chunk at 512
                ck = min(512, n - c0)
                kgc = kv_p.tile([128, HPC * 512], bf16, name="kg",
                                tag="kvg", bufs=4)
                nc.gpsimd.dma_gather(
                    out_ap=kgc[:, :HPC * ck].rearrange("p (h s) -> p h s",
                                                       h=HPC),
                    in_ap=ksrc.ap(),
                    idxs_ap=idx_sb[:, b * (S // 16) + c0 // 16:
                                   b * (S // 16) + (c0 + ck) // 16],
                    num_idxs=ck, num_idxs_reg=ck, elem_size=HDPC,
                    transpose=True)
                kgs.append((kgc, ck))
            for j in range(nj[b] if stage != 15 else 0):
                kgc, ck = kgs[j // 4]
                jl = j % 4
                for h in range(HPC):
                    col = b * HPC + h
                    nc.tensor.matmul(
                        swide_ps[:, j * 64 + col: j * 64 + col + 1],
                        kgc[:, h * ck + jl * 128: h * ck + jl * 128 + 128],
                        qT[:, col:col + 1],
                        start=True, stop=True)
            if b == 0:
                dump('kg0', kgs[0][0][:, :HPC * kgs[0][1]])
        if stage == 15:
            kf = sb_p.tile([16, 512], fp32, name="kf", tag="kf")
            nc.vector.tensor_copy(kf[:], kgs[-1][0][:16, :512])
            nc.sync.dma_start(y.ap()[:, :512], kf[:])
            return

        # ---------------- k projection + F = q . k_new ----------------------
        k_ps = pssm.tile([B, HDPC], fp32, name="k_ps", tag="sm")
        wpass(HDPC, k_ps)
        rope(k_sb, k_ps[:], kcos_sb, ksin_sb)
        kT = sb_p.tile([128, 4 * B], bf16)
        head_transpose(kT, k_sb)

        # F3[b', b*4+h] = q_{b,h} . k_new_{b',h}; FT = F3^T for the score
        # patches ([4,1] partition-aligned copies) and the p* recompute.
        F3_ps = pstp.tile([B, 4 * B], fp32, name="F3_ps", tag="tp2", bufs=1)
        for h in range(HPC):
            nc.tensor.matmul(F3_ps[:, h:4 * B:HPC],
                             kT[:, h:4 * B:HPC],
                             qT[:, h:4 * B:HPC],
                             start=True, stop=True)
        F3 = sb_p.tile([B, 4 * B], fp32)
        nc.scalar.copy(F3[:], F3_ps[:])
        FT_ps = pstp.tile([4 * B, B], fp32, name="FT_ps", tag="tp2", bufs=1)
        nc.tensor.transpose(FT_ps[:], F3[:], identf[:B, :B])
        FT = sb_p.tile([4 * B, B], fp32)
        nc.scalar.copy(FT[:], FT_ps[:])
        dump('k_sb', k_sb[:])
        dump('FT', FT[:])

        # ---------------- v projection ----------------
        v_ps = pssm.tile([B, HDPC], fp32, name="v_ps", tag="sm")
        wpass(2 * HDPC, v_ps)
        nc.scalar.copy(v_sb[:], v_ps[:])

        # wo prefetch (streams in behind the V gathers)
        wo_tiles = []
        for i in range(NOUT):
            wo = wo_p.tile([128, HPC * 512], bf16, name=f"wo{i}", tag=f"wo{i}")
            nc.sync.dma_start(
                wo[:].rearrange("p (t c) -> p t c", t=HPC),
                woT.ap()[:, i * 512:(i + 1) * 512]
                .rearrange("(t p) c -> p t c", p=128))
            wo_tiles.append(wo)

        # ---------------- softmax (single 64-row pass) ----------------------
        swide = sb_p.tile([128, max_nj * 64], fp32)
        nc.scalar.copy(swide[:], swide_ps[:])
        sc_ps = pssc.tile([64, SW], fp32, name="sc_ps", tag="sw",
                          padded_shape=[64, 1024])
        for j in range(max_nj):
            nc.tensor.transpose(sc_ps[:, j * 128:(j + 1) * 128],
                                swide[:, j * 64:(j + 1) * 64], identf[:])
        # patch the new-token scores (q.k_new, pre-scaled via q tables):
        # the patched cells hold 0 (their gather rows were redirected to the
        # zero row), so a fixm-gated rank-1 matmul add SETS them. Gating
        # keeps every other row of the touched column at +0.
        F3m = sb_p.tile([B, 4 * B], bf16)
        nc.vector.tensor_mul(F3m[:], F3[:], fixmT_sb[:])
        if plan['diag_ok']:
            for b in range(B):
                for (p, bp) in fixups[b]:
                    nc.tensor.matmul(sc_ps[:, p:p + 1], F3m[:],
                                     identb[:B, bp:bp + 1],
                                     start=False, stop=True,
                                     skip_group_check=True)
        if stage <= 2:
            nc.sync.dma_start(y.ap()[:, :512], swide[:16, :])
            return
        sc = sb_p.tile([64, SW], fp32)
        nc.vector.tensor_add(sc[:], sc_ps[:], mask_sb[:])
        dump('swide', swide[:])
        dump('sc', sc[:])
        if not plan['diag_ok']:
            for b in range(B):
                for (p, bp) in fixups[b]:
                    nc.sync.dma_start(
                        sc[b * HPC:(b + 1) * HPC, p:p + 1],
                        FT[b * HPC:(b + 1) * HPC, bp:bp + 1])
        negmax = sb_p.tile([64, 1], fp32, name="negmax", tag="nm", bufs=1)
        nc.vector.tensor_reduce(out=negmax[:], in_=sc[:],
                                axis=AX.X, op=ALU.max, negate=True)
        p_sb = sb_p.tile([64, SW], bf16)
        sums = sb_p.tile([64, 1], fp32, name="sums", tag="sums", bufs=1)
        nc.scalar.activation(out=p_sb[:], in_=sc[:],
                             func=ACTF.Exp, bias=negmax[:],
                             scale=1.0, accum_out=sums[:])
        rsum = sb_p.tile([64, 1], fp32, name="rsum", tag="rs", bufs=1)
        nc.vector.reciprocal(rsum[:], sums[:])
        nc.vector.tensor_scalar_mul(p_sb[:], p_sb[:], rsum[:])
        dump('p_sb', p_sb[:])

        # pT[s, j*64 + b*4+h] for the V-phase matmuls
        pT = sb_p.tile([128, max_nj * 64], bf16)
        for j in range(max_nj):
            pt_ps = pstp.tile([128, 64], bf16, name="pt_ps", tag="tp")
            nc.tensor.transpose(pt_ps[:], p_sb[:, j * 128:(j + 1) * 128],
                                identb[:64, :64])
            nc.scalar.copy(pT[:, j * 64:(j + 1) * 64], pt_ps[:])
        dump('pT', pT[:])
        if stage <= 3:
            nc.sync.dma_start(y.ap()[:, :SW], sc[:16, :])
            return

        # p*[b', b*4+h] = exp(F - max) * rsum, gated to real fixup pairs;
        # transposed for the rank-1 correction matmuls.
        p2 = sb_p.tile([64, B], fp32)
        nc.scalar.activation(out=p2[:], in_=FT[:], func=ACTF.Exp,
                             bias=negmax[:], scale=1.0)
        nc.vector.tensor_scalar_mul(p2[:], p2[:], rsum[:])
        p2m = sb_p.tile([64, B], bf16)
        nc.vector.tensor_mul(p2m[:], p2[:], fixm_sb[:])
        pTx_ps = pstp.tile([B, 4 * B], bf16, name="pTx_ps", tag="tp")
        nc.tensor.transpose(pTx_ps[:], p2m[:], identb[:64, :64])
        pTx = sb_p.tile([B, 4 * B], bf16)
        nc.scalar.copy(pTx[:], pTx_ps[:])
        dump('v_sb', v_sb[:])
        dump('pTx', pTx[:])

        # ---------------- V phase -------------------------------------------
        # attnT[d, h*16+b] = sum_j vg_j^T p_j, plus the rank-1 new-token
        # correction (overwritten rows were gathered as zeros).
        atT_ps = psat.tile([128, 64], fp32)      # cols h*16+b
        for b in range(B):
            vg = vg_p.tile([128, njv[b] * HDPC], bf16, name=f"vg{b}",
                           tag=f"vg{b}")
            n = njv[b] * 128   # full chunks; pad indices read the zero row
            nc.gpsimd.dma_gather(
                out_ap=vg[:].rearrange("p (j e) -> p j e",
                                       e=HDPC)[:, :njv[b], :],
                in_ap=vsrc.ap(),
                idxs_ap=idx_sb[:, b * (S // 16): b * (S // 16) + n // 16],
                num_idxs=n, num_idxs_reg=n, elem_size=HDPC)
            if b == 1:
                dump('vg1', vg[:])
            for h in range(HPC):
                col = h * B + b
                for j in range(njv[b]):
                    nc.tensor.matmul(
                        atT_ps[:, col:col + 1],
                        vg[:, j * HDPC + h * HD: j * HDPC + (h + 1) * HD],
                        pT[:, j * 64 + b * HPC + h:
                           j * 64 + b * HPC + h + 1],
                        start=(j == 0), stop=(j == njv[b] - 1))
        C_ps = pssm.tile([128, 64], fp32, name="C_ps", tag="sm")
        for h in range(HPC):
            nc.tensor.matmul(C_ps[:, h * B:(h + 1) * B],
                             v_sb[:, h * HD:(h + 1) * HD],
                             pTx[:, h:4 * B:HPC],
                             start=True, stop=True)
        C_sb = sb_p.tile([128, 64], fp32)
        nc.scalar.copy(C_sb[:], C_ps[:])
        attnT = sb_p.tile([128, 64], bf16)
        nc.vector.tensor_add(attnT[:], atT_ps[:], C_sb[:])
        dump('attnT', attnT[:])
        if stage <= 4:
            nc.sync.dma_start(y.ap()[:, :64], C_sb[:16, :])
            return

        # ------------- output projection (4 chunks packed per PE pass) ------
        for r in range(NOUT // 4):
            yt = psyt.tile([128, 512], fp32, name="yt", tag="yt")
            nc.vector.memset(yt[:], 0)
            for c in range(4):
                i = r * 4 + c
                for h in range(HPC):
                    nc.tensor.matmul(yt[32 * c:32 * c + B, :],
                                     attnT[:, h * B:(h + 1) * B],
                                     wo_tiles[i][:, h * 512:(h + 1) * 512],
                                     start=(h == 0), stop=(h == HPC - 1),
                                     tile_position=(0, 32 * c))
            yst = tmp_p.tile([128, 512], fp32, name="yst", tag="yst")
            nc.scalar.copy(yst[:], yt[:])
            for c in range(4):
                i = r * 4 + c
                nc.sync.dma_start(y.ap()[:, i * 512:(i + 1) * 512],
                                  yst[32 * c:32 * c + B, :])

    _body()
    nc.compile()
    return nc


def _make_in_maps(hidden_states, qkv_w, out_w, key_cache, value_cache, plan):
    from ml_dtypes import bfloat16

    hid = hidden_states.reshape(B, HID).astype(np.float32)
    hiddenT = np.ascontiguousarray(hid.T).astype(bfloat16)

    wq, wk, wv = qkv_w[:HID], qkv_w[HID:2 * HID], qkv_w[2 * HID:]
    kc = key_cache.reshape(NSLOT, NH, HD)
    vc = value_cache.reshape(NSLOT, NH, HD)

    zrow = np.zeros((1, HDPC), dtype=bfloat16)
    in_maps = []
    for c in range(N_CORES):
        h0 = c * HPC
        r0, r1 = h0 * HD, (h0 + HPC) * HD
        wqkvT = np.ascontiguousarray(
            np.concatenate([wq[r0:r1], wk[r0:r1], wv[r0:r1]],
                           axis=0).T).astype(bfloat16)
        woT = np.ascontiguousarray(out_w[:, r0:r1].T).astype(bfloat16)
        ks = np.concatenate(
            [np.ascontiguousarray(
                kc[:, h0:h0 + HPC, :].reshape(NSLOT, HDPC)).astype(bfloat16),
             zrow], axis=0)
        vs = np.concatenate(
            [np.ascontiguousarray(
                vc[:, h0:h0 + HPC, :].reshape(NSLOT, HDPC)).astype(bfloat16),
             zrow], axis=0)
        in_maps.append({
            "hiddenT": hiddenT, "wqkvT": wqkvT, "woT": woT,
            "ksrc": ks, "vsrc": vs,
            "idxs": plan['idx_tiles'], "maskd": plan['mask'],
            "fixmd": plan['fixm'], "fixmTd": plan['fixmT'],
            "qcosd": plan['qcos'], "qsind": plan['qsin'],
            "kcosd": plan['cos_rep'], "ksind": plan['sin_rep'],
        })
    return in_maps


def kernel(hidden_states, qkv_w, out_w, cos_sin_cache, key_cache, value_cache,
           position_ids, block_tables, slots, context_lengths):
    from concourse.bass_utils import run_bass_kernel_spmd

    hidden_states = np.asarray(hidden_states, dtype=np.float32)
    qkv_w = np.asarray(qkv_w, dtype=np.float32)
    out_w = np.asarray(out_w, dtype=np.float32)
    cos_sin_cache = np.asarray(cos_sin_cache, dtype=np.float32)
    key_cache = np.asarray(key_cache, dtype=np.float32)
    value_cache = np.asarray(value_cache, dtype=np.float32)

    plan = _plan(np.asarray(position_ids), np.asarray(block_tables),
                 np.asarray(slots), np.asarray(context_lengths), cos_sin_cache)
    nc = _build_bass(plan)
    in_maps = _make_in_maps(hidden_states, qkv_w, out_w, key_cache,
                            value_cache, plan)

    res = run_bass_kernel_spmd(nc, in_maps, core_ids=list(range(N_CORES)))
    out = np.zeros((B, HID), dtype=np.float32)
    for c in range(N_CORES):
        out += res.results[c]["y"]
    return out.reshape(B, 1, HID)


# revision 82
# speedup vs baseline: 3.2620x; 1.2019x over previous
"""Paged attention with RoPE (decode, B=16, L=1) on 8 trn2 NeuronCores.

Sharding: tensor-parallel over heads. 32 heads / 8 cores = 4 heads per core.
Per core: QKV projection for its heads, RoPE, paged attention over its head
shard of the kv cache, partial output projection; host sums the 8 partials.

The kernel is engineered around the serialized DMA stream (the roofline
resource): all bulk traffic (weights, KV gathers) moves in bf16, halving the
byte count vs fp32. K is gathered with dma_gather(transpose=True) so the
head dim lands on partitions and attention scores become tensor-engine
matmuls (N=1 per 128-token chunk) instead of vector-engine mul+reduce.
Softmax runs in a single 64-row pass ([b*4+h, S]).

The reference's reshape_and_cache scatter is never materialized: gather
indices of cache slots that the new tokens overwrite are redirected to a
zero row appended to the K/V sources. The new-token K scores are patched
into the score tile with partition-aligned vector copies from an on-chip
F = q . k_new matrix, and the new-token V contribution is added as
PSUM-accumulated rank-1 matmuls using p* = exp(F - max) / sum recomputed
from the same softmax statistics.
"""

import numpy as np

B = 16
HID = 4096
NH = 32
HD = 128
BS = 16
MAXB = 64
NB = 1024
S = MAXB * BS          # 1024 max context
NSLOT = NB * BS        # 16384
ZROW = NSLOT           # index of the appended all-zero cache row
N_CORES = 8
HPC = NH // N_CORES    # 4 heads per core
HDPC = HPC * HD        # 512 elements of head-dim per core
KTILES = HID // 128    # 32
NOUT = HID // 512      # 8 output-projection chunks
SCALE = 1.0 / float(np.sqrt(HD))
NEG = -1.0e30


def _plan(position_ids, block_tables, slots, context_lengths, cos_sin_cache):
    """Host-side planning shared by all cores: gather indices, fixups, masks,
    rope tables, per-sequence tile counts."""
    pos = np.clip(position_ids.reshape(B).astype(np.int64), 0,
                  cos_sin_cache.shape[0] - 1)
    bt = block_tables.astype(np.int64)          # [B, MAXB]
    sl = slots.astype(np.int64)                 # [B]
    ctx = np.maximum(context_lengths.astype(np.int64), 1)  # reference: ctx>=1

    # K gathers (transpose mode) need multiples of 128; V gathers of 16.
    n128 = ((ctx + 127) // 128 * 128).astype(np.int64)
    n16 = ((ctx + 15) // 16 * 16).astype(np.int64)
    nj = (n128 // 128).astype(np.int64)
    njv = ((n16 + 127) // 128).astype(np.int64)
    max_nj = int(nj.max())

    p_all = np.arange(S, dtype=np.int64)
    slot_all = bt[:, p_all // BS] * BS + (p_all % BS)      # [B, S]

    # new-token overwrite map: slot -> writing sequence (slots are unique)
    fix = np.full((NSLOT,), -1, dtype=np.int64)
    for bp in range(B):
        fix[sl[bp]] = bp

    fixups = []   # per b: list of (gathered position p, source sequence b')
    for b in range(B):
        fixups.append([(int(p), int(fix[slot_all[b, p]]))
                       for p in range(int(ctx[b]))
                       if fix[slot_all[b, p]] >= 0])

    # concatenated int16 gather index stream (one 128-token chunk per
    # sequence-slice, b-major), wrapped mod 16 partitions, replicated x8.
    # Positions >= ctx (padding) and overwritten slots read the zero row.
    # Gathers are bin-packed over this stream: K (transpose mode) bins of
    # <= 7 chunks (the hw caps transpose gathers below 1024 idxs), V bins
    # of <= 8 chunks.
    gstart = np.concatenate([[0], np.cumsum(nj)]).astype(np.int64)
    ncat = int(gstart[-1])                       # total 128-token chunks
    cat = np.zeros(ncat * 128, dtype=np.int64)
    for b in range(B):
        n = int(n128[b])
        idx = slot_all[b, :n].copy()
        idx[int(ctx[b]):] = ZROW
        for (p, _) in fixups[b]:
            idx[p] = ZROW
        cat[int(gstart[b]) * 128: int(gstart[b]) * 128 + n] = idx
    wrapped = cat.reshape(-1, 16).T              # [16, ncat*8]
    catidx = np.ascontiguousarray(
        np.tile(wrapped, (8, 1)).astype(np.int16))

    def pack(cap):
        bins, g0 = [], 0
        while g0 < ncat:
            bins.append((g0, min(g0 + cap, ncat)))
            g0 = min(g0 + cap, ncat)
        return bins
    k_bins = pack(7)
    v_bins = pack(8)

    # additive mask [64 rows = b*HPC+h, max context]
    mask = np.zeros((64, S), dtype=np.float32)
    for b in range(B):
        mask[b * HPC:(b + 1) * HPC, int(ctx[b]):] = NEG
    mask = np.ascontiguousarray(mask[:, :max_nj * 128])

    # 1.0 where (b, b') is a real fixup pair, else 0 — gates the rank-1
    # new-token V correction (rows b*4+h, cols b') and, transposed, the
    # score-patch matmuls (rows b', cols b*4+h).
    fixm = np.zeros((64, B), dtype=np.float32)
    for b in range(B):
        for (_, bp) in fixups[b]:
            fixm[b * HPC:(b + 1) * HPC, bp] = 1.0

    # patches are applied as PE matmuls gated by fixm^T; that only stays
    # collision-free when each writer b' patches a single sequence (true
    # whenever block tables are disjoint, which the reference guarantees —
    # every fixup is then (b, ctx_b-1, b)). Cross fixups fall back to DMA.
    writers = [bp for fb in fixups for (_, bp) in fb]
    diag_ok = len(writers) == len(set(writers))

    # rope tables, per-head replicated, sin sign baked ( [-sin | +sin] );
    # the q tables additionally fold in the 1/sqrt(HD) score scale.
    cs = cos_sin_cache[pos]                     # [B, 128]
    cos_h, sin_h = cs[:, :64], cs[:, 64:]
    cos_full = np.concatenate([cos_h, cos_h], axis=1)
    sin_sign = np.concatenate([-sin_h, sin_h], axis=1)
    cos_rep = np.ascontiguousarray(np.tile(cos_full, (1, HPC)), dtype=np.float32)
    sin_rep = np.ascontiguousarray(np.tile(sin_sign, (1, HPC)), dtype=np.float32)

    # q/k projections run in fp8 with hidden and weights both scaled x16 on
    # the host; the 1/256 comes back out through the rope tables.
    FP8S = 1.0 / 256.0
    return {
        'nj': [int(x) for x in nj], 'gstart': [int(x) for x in gstart],
        'ncat': ncat, 'k_bins': k_bins, 'v_bins': v_bins,
        'max_nj': max_nj, 'fixups': fixups,
        'catidx': catidx, 'mask': mask, 'fixm': fixm,
        'fixmT': np.ascontiguousarray(fixm.T), 'diag_ok': diag_ok,
        'cos_rep': cos_rep * FP8S, 'sin_rep': sin_rep * FP8S,
        'qcos': cos_rep * (SCALE * FP8S), 'qsin': sin_rep * (SCALE * FP8S),
    }


def _build_bass(plan, debug=False, stage=99):
    """Build the per-core bass program (identical program for every core;
    only the input data differs)."""
    import concourse.tile as tile
    from concourse import bacc, mybir
    from concourse.masks import make_identity
    from contextlib import ExitStack

    fp32 = mybir.dt.float32
    bf16 = mybir.dt.bfloat16
    i16 = mybir.dt.int16
    AX = mybir.AxisListType
    ALU = mybir.AluOpType
    ACTF = mybir.ActivationFunctionType

    nj = plan['nj']
    gstart = plan['gstart']
    ncat = plan['ncat']
    k_bins = plan['k_bins']
    v_bins = plan['v_bins']
    max_nj = plan['max_nj']
    fixups = plan['fixups']
    SW = max_nj * 128          # score width

    # 32KB descriptor ring + 2 SWDGE queues let gather descriptor
    # generation run ahead of the transfers instead of ping-ponging.
    nc = bacc.Bacc("TRN2", target_bir_lowering=False, debug=False,
                   num_devices=N_CORES, dynamic_dma_scratch_size=32768)

    fp8 = mybir.dt.float8e4

    # hidden pre-packed on host into the SBUF tile layout [p, (kt, b)] —
    # the natural [HID, B] layout DMAs as 16-element descriptors (slow).
    hiddenT = nc.dram_tensor("hiddenT", [128, KTILES * B], bf16,
                             kind="ExternalInput")
    hidden8 = nc.dram_tensor("hidden8", [128, KTILES * B], fp8,
                             kind="ExternalInput")
    wqkT8 = nc.dram_tensor("wqkT8", [HID, 2 * HDPC], fp8, kind="ExternalInput")
    wvT = nc.dram_tensor("wvT", [HID, HDPC], bf16, kind="ExternalInput")
    woT = nc.dram_tensor("woT", [HDPC, HID], bf16, kind="ExternalInput")
    ksrc = nc.dram_tensor("ksrc", [NSLOT + 1, HDPC], bf16, kind="ExternalInput")
    vsrc = nc.dram_tensor("vsrc", [NSLOT + 1, HDPC], bf16, kind="ExternalInput")
    idxs = nc.dram_tensor("idxs", [128, ncat * 8], i16, kind="ExternalInput")
    maskd = nc.dram_tensor("maskd", [64, SW], fp32, kind="ExternalInput")
    fixmd = nc.dram_tensor("fixmd", [64, B], fp32, kind="ExternalInput")
    fixmTd = nc.dram_tensor("fixmTd", [B, 64], fp32, kind="ExternalInput")
    qcosd = nc.dram_tensor("qcosd", [B, HDPC], fp32, kind="ExternalInput")
    qsind = nc.dram_tensor("qsind", [B, HDPC], fp32, kind="ExternalInput")
    kcosd = nc.dram_tensor("kcosd", [B, HDPC], fp32, kind="ExternalInput")
    ksind = nc.dram_tensor("ksind", [B, HDPC], fp32, kind="ExternalInput")
    # partial output in raw PE layout: y[p, c*16+b] = out[b, c*128+p];
    # the host untangles (cheaper than on-chip transposes).
    y = nc.dram_tensor("y", [128, 512], fp32, kind="ExternalOutput")
    dump_list = []

    def dump(name, ap):
        if not debug:
            return
        d = nc.dram_tensor(f"dbg_{name}", list(ap.shape), ap.dtype,
                           kind="ExternalOutput")
        nc.sync.dma_start(d.ap(), ap)
        dump_list.append(name)

    def _body():
      with tile.TileContext(nc) as tc, ExitStack() as ctx:
        const_p = ctx.enter_context(tc.tile_pool(name="const", bufs=1))
        w_p = ctx.enter_context(tc.tile_pool(name="w", bufs=2))
        wo_p = ctx.enter_context(tc.tile_pool(name="wop", bufs=1))
        kv_p = ctx.enter_context(tc.tile_pool(name="kv", bufs=3))
        vg_p = ctx.enter_context(tc.tile_pool(name="vgp", bufs=1))
        sb_p = ctx.enter_context(tc.tile_pool(name="sb", bufs=1))
        tmp_p = ctx.enter_context(tc.tile_pool(name="tmp", bufs=2))
        pssc = ctx.enter_context(tc.tile_pool(name="pssc", bufs=1, space="PSUM"))
        pssm = ctx.enter_context(tc.tile_pool(name="pssm", bufs=1, space="PSUM"))
        pstp = ctx.enter_context(tc.tile_pool(name="pstp", bufs=2, space="PSUM"))
        psat = ctx.enter_context(tc.tile_pool(name="psat", bufs=1, space="PSUM"))
        psyt = ctx.enter_context(tc.tile_pool(name="psyt", bufs=1, space="PSUM"))

        # ---------------- constants ----------------
        # idx first: the K gathers (the bulk of early DMA) wait only on it
        idx_sb = const_p.tile([128, ncat * 8], i16)
        nc.sync.dma_start(idx_sb[:], idxs.ap())
        identf = const_p.tile([128, 128], fp32)
        make_identity(nc, identf[:])
        identb = const_p.tile([128, 128], bf16)
        nc.vector.tensor_copy(identb[:], identf[:])
        ht_sb = const_p.tile([128, KTILES * B], bf16)
        nc.sync.dma_start(ht_sb[:], hiddenT.ap())
        h8_sb = const_p.tile([128, KTILES * B], fp8)
        nc.sync.dma_start(h8_sb[:], hidden8.ap())
        qcos_sb = const_p.tile([B, HDPC], fp32)
        nc.sync.dma_start(qcos_sb[:], qcosd.ap())
        qsin_sb = const_p.tile([B, HDPC], fp32)
        nc.sync.dma_start(qsin_sb[:], qsind.ap())
        kcos_sb = const_p.tile([B, HDPC], fp32)
        nc.sync.dma_start(kcos_sb[:], kcosd.ap())
        ksin_sb = const_p.tile([B, HDPC], fp32)
        nc.sync.dma_start(ksin_sb[:], ksind.ap())
        mask_sb = const_p.tile([64, SW], fp32)
        nc.sync.dma_start(mask_sb[:], maskd.ap())
        fixm_sb = const_p.tile([64, B], fp32)
        nc.sync.dma_start(fixm_sb[:], fixmd.ap())
        fixmT_sb = const_p.tile([B, 64], fp32)
        nc.sync.dma_start(fixmT_sb[:], fixmTd.ap())

        def rope(dst, src, cos_sb, sin_sb):
            src3 = src.rearrange("b (h two d) -> b h two d", two=2, d=64)
            rot = tmp_p.tile([B, HDPC], fp32, name="rot", tag="rot", bufs=1)
            rot3 = rot[:].rearrange("b (h two d) -> b h two d", two=2, d=64)
            nc.vector.tensor_copy(rot3[:, :, 0, :], src3[:, :, 1, :])
            nc.vector.tensor_copy(rot3[:, :, 1, :], src3[:, :, 0, :])
            nc.vector.tensor_mul(rot[:], rot[:], sin_sb[:])
            cp = tmp_p.tile([B, HDPC], fp32, name="cp", tag="cp", bufs=1)
            nc.vector.tensor_mul(cp[:], src, cos_sb[:])
            nc.vector.tensor_add(dst[:], cp[:], rot[:])

        q_sb = sb_p.tile([B, HDPC], bf16)
        k_sb = sb_p.tile([B, HDPC], bf16)
        v_sb = sb_p.tile([B, HDPC], bf16)

        WCHUNK = 8  # weight k-tiles per DMA

        def wpass(col0, out_ps):
            # q/k projections: fp8 weights and hidden (x16-scaled on host;
            # the 1/256 is folded into the rope tables)
            for wc in range(KTILES // WCHUNK):
                wt = w_p.tile([128, WCHUNK * HDPC], fp8, name="wt", tag="w")
                nc.sync.dma_start(
                    wt[:].rearrange("p (t c) -> p t c", t=WCHUNK),
                    wqkT8.ap()[wc * WCHUNK * 128:(wc + 1) * WCHUNK * 128,
                               col0:col0 + HDPC]
                    .rearrange("(t p) c -> p t c", p=128))
                for t in range(WCHUNK):
                    kt = wc * WCHUNK + t
                    nc.tensor.matmul(out_ps[:],
                                     h8_sb[:, kt * B:(kt + 1) * B],
                                     wt[:, t * HDPC:(t + 1) * HDPC],
                                     start=(kt == 0), stop=(kt == KTILES - 1))

        def wpass_v(out_ps):
            for wc in range(KTILES // WCHUNK):
                wt = w_p.tile([128, WCHUNK * HDPC], bf16, name="wtv", tag="wv")
                nc.sync.dma_start(
                    wt[:].rearrange("p (t c) -> p t c", t=WCHUNK),
                    wvT.ap()[wc * WCHUNK * 128:(wc + 1) * WCHUNK * 128, :]
                    .rearrange("(t p) c -> p t c", p=128))
                for t in range(WCHUNK):
                    kt = wc * WCHUNK + t
                    nc.tensor.matmul(out_ps[:],
                                     ht_sb[:, kt * B:(kt + 1) * B],
                                     wt[:, t * HDPC:(t + 1) * HDPC],
                                     start=(kt == 0), stop=(kt == KTILES - 1))

        # transpose a [B, HDPC] sbuf tile into dstT [128, 64] (cols b*4+h)
        def head_transpose(dstT, src_sb):
            for h in range(HPC):
                tp = pstp.tile([128, B], bf16, name="tp", tag="tp")
                nc.tensor.transpose(tp[:], src_sb[:, h * HD:(h + 1) * HD],
                                    identb[:B, :B])
                nc.scalar.copy(dstT[:, h:4 * B:HPC], tp[:])

        # ---------------- q projection (first, to unblock scores) ----------
        # tile_wait_until hints order the weight streams explicitly:
        # q8 -> k8 -> wv -> wo -> V gathers; the scheduler's own choices
        # interleave them badly.
        q_ps = pssm.tile([B, HDPC], fp32, name="q_ps", tag="sm")
        with tc.tile_wait_until(0.001):
            wpass(0, q_ps)
        rope(q_sb, q_ps[:], qcos_sb, qsin_sb)   # scale baked into q tables
        qT = sb_p.tile([128, 4 * B], bf16)      # [d, b*4+h]
        head_transpose(qT, q_sb)
        dump('q_sb', q_sb[:])
        dump('qT', qT[:])
        if stage <= 1:
            qf = sb_p.tile([B, HDPC], fp32, name="qf", tag="qf")
            nc.vector.tensor_copy(qf[:], q_sb[:])
            nc.sync.dma_start(y.ap()[:16, :], qf[:, :512])
            return

        # ---------------- K gather (transposed) + scores --------------------
        # kgT[b]: [128 d, 4 h, nj*128 s] via transpose-mode gather; score
        # s[token, b*4+h] = kgT_h^T . qT col — N=1 matmuls into PSUM swide.
        swide_ps = pssc.tile([128, max_nj * 64], fp32, name="swide", tag="sw",
                             padded_shape=[128, 512])
        nc.vector.memset(swide_ps[:], 0)
        kbin_tiles = []    # (tile, g0, nbin_tokens)
        for i, (g0, g1) in enumerate(k_bins):
            nbin = (g1 - g0) * 128
            kgc = kv_p.tile([128, HPC * nbin], bf16, name="kg",
                            tag="kvg", bufs=4, padded_shape=[128, HPC * 896])
            nc.gpsimd.dma_gather(
                out_ap=kgc[:, :HPC * nbin].rearrange("p (h s) -> p h s",
                                                     h=HPC),
                in_ap=ksrc.ap(),
                idxs_ap=idx_sb[:, g0 * 8:g1 * 8],
                num_idxs=nbin, num_idxs_reg=nbin, elem_size=HDPC,
                transpose=True)
            kbin_tiles.append((kgc, g0, nbin))

        def kbin_of(g):
            for (t, g0, nbin) in kbin_tiles:
                if g0 <= g < g0 + nbin // 128:
                    return t, g - g0, nbin
            raise AssertionError(g)

        if stage != 15:
            for b in range(B):
                for j in range(nj[b]):
                    kgc, off, nbin = kbin_of(gstart[b] + j)
                    for h in range(HPC):
                        col = b * HPC + h
                        nc.tensor.matmul(
                            swide_ps[:, j * 64 + col: j * 64 + col + 1],
                            kgc[:, h * nbin + off * 128:
                                h * nbin + off * 128 + 128],
                            qT[:, col:col + 1],
                            start=True, stop=True)
        if stage == 15:
            kf = sb_p.tile([16, 512], fp32, name="kf", tag="kf")
            nc.vector.tensor_copy(kf[:], kbin_tiles[-1][0][:16, :512])
            nc.sync.dma_start(y.ap()[:16, :], kf[:])
            return

        # ---------------- k projection + F = q . k_new ----------------------
        k_ps = pssm.tile([B, HDPC], fp32, name="k_ps", tag="sm")
        with tc.tile_wait_until(0.002):
            wpass(HDPC, k_ps)   # cols HDPC..2*HDPC of wqkT8 = k shard
        rope(k_sb, k_ps[:], kcos_sb, ksin_sb)
        kT = sb_p.tile([128, 4 * B], bf16)
        head_transpose(kT, k_sb)

        # F3[b', b*4+h] = q_{b,h} . k_new_{b',h}; FT = F3^T for the score
        # patches ([4,1] partition-aligned copies) and the p* recompute.
        F3_ps = pstp.tile([B, 4 * B], fp32, name="F3_ps", tag="tp2", bufs=1)
        for h in range(HPC):
            nc.tensor.matmul(F3_ps[:, h:4 * B:HPC],
                             kT[:, h:4 * B:HPC],
                             qT[:, h:4 * B:HPC],
                             start=True, stop=True)
        F3 = sb_p.tile([B, 4 * B], fp32)
        nc.scalar.copy(F3[:], F3_ps[:])
        FT_ps = pstp.tile([4 * B, B], fp32, name="FT_ps", tag="tp2", bufs=1)
        nc.tensor.transpose(FT_ps[:], F3[:], identf[:B, :B])
        FT = sb_p.tile([4 * B, B], fp32)
        nc.scalar.copy(FT[:], FT_ps[:])
        dump('k_sb', k_sb[:])
        dump('FT', FT[:])

        # ---------------- v projection ----------------
        v_ps = pssm.tile([B, HDPC], fp32, name="v_ps", tag="sm")
        with tc.tile_wait_until(0.003):
            wpass_v(v_ps)
        nc.scalar.copy(v_sb[:], v_ps[:])

        # wo prefetch (streams in behind the V gathers)
        wo_tiles = []
        with tc.tile_wait_until(0.035):
          for i in range(NOUT):
            wo = wo_p.tile([128, HPC * 512], bf16, name=f"wo{i}",
                           tag=f"wo{i}")
            nc.sync.dma_start(
                wo[:].rearrange("p (t c) -> p t c", t=HPC),
                woT.ap()[:, i * 512:(i + 1) * 512]
                .rearrange("(t p) c -> p t c", p=128))
            wo_tiles.append(wo)

        # ---------------- softmax (single 64-row pass) ----------------------
        swide = sb_p.tile([128, max_nj * 64], fp32)
        nc.scalar.copy(swide[:], swide_ps[:])
        sc_ps = pssc.tile([64, SW], fp32, name="sc_ps", tag="sw",
                          padded_shape=[64, 1024])
        for j in range(max_nj):
            nc.tensor.transpose(sc_ps[:, j * 128:(j + 1) * 128],
                                swide[:, j * 64:(j + 1) * 64], identf[:])
        # patch the new-token scores (q.k_new, pre-scaled via q tables):
        # the patched cells hold 0 (their gather rows were redirected to the
        # zero row), so a fixm-gated rank-1 matmul add SETS them. Gating
        # keeps every other row of the touched column at +0.
        F3m = sb_p.tile([B, 4 * B], bf16)
        nc.vector.tensor_mul(F3m[:], F3[:], fixmT_sb[:])
        if plan['diag_ok']:
            for b in range(B):
                for (p, bp) in fixups[b]:
                    nc.tensor.matmul(sc_ps[:, p:p + 1], F3m[:],
                                     identb[:B, bp:bp + 1],
                                     start=False, stop=True,
                                     skip_group_check=True)
        if stage <= 2:
            nc.sync.dma_start(y.ap()[:16, :], swide[:16, :])
            return
        sc = sb_p.tile([64, SW], fp32)
        nc.vector.tensor_add(sc[:], sc_ps[:], mask_sb[:])
        dump('swide', swide[:])
        dump('sc', sc[:])
        if not plan['diag_ok']:
            for b in range(B):
                for (p, bp) in fixups[b]:
                    nc.sync.dma_start(
                        sc[b * HPC:(b + 1) * HPC, p:p + 1],
                        FT[b * HPC:(b + 1) * HPC, bp:bp + 1])
        negmax = sb_p.tile([64, 1], fp32, name="negmax", tag="nm", bufs=1)
        nc.vector.tensor_reduce(out=negmax[:], in_=sc[:],
                                axis=AX.X, op=ALU.max, negate=True)
        p_sb = sb_p.tile([64, SW], bf16)
        sums = sb_p.tile([64, 1], fp32, name="sums", tag="sums", bufs=1)
        nc.scalar.activation(out=p_sb[:], in_=sc[:],
                             func=ACTF.Exp, bias=negmax[:],
                             scale=1.0, accum_out=sums[:])
        rsum = sb_p.tile([64, 1], fp32, name="rsum", tag="rs", bufs=1)
        nc.vector.reciprocal(rsum[:], sums[:])
        nc.vector.tensor_scalar_mul(p_sb[:], p_sb[:], rsum[:])
        dump('p_sb', p_sb[:])

        # pT[s, j*64 + b*4+h] for the V-phase matmuls
        pT = sb_p.tile([128, max_nj * 64], bf16)
        for j in range(max_nj):
            pt_ps = pstp.tile([128, 64], bf16, name="pt_ps", tag="tp")
            nc.tensor.transpose(pt_ps[:], p_sb[:, j * 128:(j + 1) * 128],
                                identb[:64, :64])
            nc.scalar.copy(pT[:, j * 64:(j + 1) * 64], pt_ps[:])
        dump('pT', pT[:])
        if stage <= 3:
            nc.sync.dma_start(y.ap()[:16, :], sc[:16, :512])
            return

        # p*[b', b*4+h] = exp(F - max) * rsum, gated to real fixup pairs;
        # transposed for the rank-1 correction matmuls.
        p2 = sb_p.tile([64, B], fp32)
        nc.scalar.activation(out=p2[:], in_=FT[:], func=ACTF.Exp,
                             bias=negmax[:], scale=1.0)
        nc.vector.tensor_scalar_mul(p2[:], p2[:], rsum[:])
        p2m = sb_p.tile([64, B], bf16)
        nc.vector.tensor_mul(p2m[:], p2[:], fixm_sb[:])
        pTx_ps = pstp.tile([B, 4 * B], bf16, name="pTx_ps", tag="tp")
        nc.tensor.transpose(pTx_ps[:], p2m[:], identb[:64, :64])
        pTx = sb_p.tile([B, 4 * B], bf16)
        nc.scalar.copy(pTx[:], pTx_ps[:])
        dump('v_sb', v_sb[:])
        dump('pTx', pTx[:])

        # ---------------- V phase -------------------------------------------
        # attnT[d, h*16+b] = sum_j vg_j^T p_j, plus the rank-1 new-token
        # correction (overwritten rows were gathered as zeros).
        atT_ps = psat.tile([128, 64], fp32)      # cols h*16+b
        # tile_wait_until keeps the scheduler from hoisting these ahead of
        # the K gathers on the Pool stream (K finishing late delays softmax
        # and with it everything downstream).
        vbin_tiles = []    # (tile, g0, nchunks)
        with tc.tile_wait_until(0.05):
            for i, (g0, g1) in enumerate(v_bins):
                n = (g1 - g0) * 128
                vg = vg_p.tile([128, (g1 - g0) * HDPC], bf16, name="vg",
                               tag="vgb", bufs=6,
                               padded_shape=[128, 8 * HDPC])
                nc.gpsimd.dma_gather(
                    out_ap=vg[:, :(g1 - g0) * HDPC]
                    .rearrange("p (j e) -> p j e", e=HDPC),
                    in_ap=vsrc.ap(),
                    idxs_ap=idx_sb[:, g0 * 8:g1 * 8],
                    num_idxs=n, num_idxs_reg=n, elem_size=HDPC)
                vbin_tiles.append((vg, g0, g1 - g0))

        def vbin_of(g):
            for (t, g0, nch) in vbin_tiles:
                if g0 <= g < g0 + nch:
                    return t, g - g0
            raise AssertionError(g)

        for b in range(B):
            for h in range(HPC):
                col = h * B + b
                for j in range(nj[b]):
                    vg, off = vbin_of(gstart[b] + j)
                    nc.tensor.matmul(
                        atT_ps[:, col:col + 1],
                        vg[:, off * HDPC + h * HD: off * HDPC + (h + 1) * HD],
                        pT[:, j * 64 + b * HPC + h:
                           j * 64 + b * HPC + h + 1],
                        start=(j == 0), stop=(j == nj[b] - 1))
        C_ps = pssm.tile([128, 64], fp32, name="C_ps", tag="sm")
        for h in range(HPC):
            nc.tensor.matmul(C_ps[:, h * B:(h + 1) * B],
                             v_sb[:, h * HD:(h + 1) * HD],
                             pTx[:, h:4 * B:HPC],
                             start=True, stop=True)
        C_sb = sb_p.tile([128, 64], fp32)
        nc.scalar.copy(C_sb[:], C_ps[:])
        attnT = sb_p.tile([128, 64], bf16)
        nc.vector.tensor_add(attnT[:], atT_ps[:], C_sb[:])
        dump('attnT', attnT[:])
        if stage <= 4:
            nc.sync.dma_start(y.ap()[:16, :64], C_sb[:16, :])
            return

        # ------------- output projection (flipped: wo stationary) -----------
        # yT[p, c*16+b] = sum_{h,dd} wo[(h,dd), c*128+p] attnT[dd, h*16+b]
        # — N=16 moving rows per matmul, so the whole projection costs ~1 us
        # of PE time at the tail. The host untangles the layout.
        yT = psyt.tile([128, 512], fp32, name="yT", tag="yt")
        for c in range(32):
            i, cc = c // 4, c % 4
            for h in range(HPC):
                nc.tensor.matmul(
                    yT[:, c * B:(c + 1) * B],
                    wo_tiles[i][:, h * 512 + cc * 128: h * 512 + cc * 128 + 128],
                    attnT[:, h * B:(h + 1) * B],
                    start=(h == 0), stop=(h == HPC - 1))
        yst = tmp_p.tile([128, 512], fp32, name="yst", tag="yst")
        nc.scalar.copy(yst[:], yT[:])
        nc.sync.dma_start(y.ap(), yst[:])

    _body()
    nc.compile()
    return nc


def _make_in_maps(hidden_states, qkv_w, out_w, key_cache, value_cache, plan):
    from ml_dtypes import bfloat16, float8_e4m3

    hid = hidden_states.reshape(B, HID).astype(np.float32)
    # pack [p, (kt, b)]: ht[p, t*B+b] = hidden[b, t*128+p]
    hp = np.ascontiguousarray(
        hid.T.reshape(KTILES, 128, B).transpose(1, 0, 2).reshape(128, KTILES * B))
    hiddenT = hp.astype(bfloat16)
    hidden8 = (hp * 16.0).astype(float8_e4m3)

    wq, wk, wv = qkv_w[:HID], qkv_w[HID:2 * HID], qkv_w[2 * HID:]
    kc = key_cache.reshape(NSLOT, NH, HD)
    vc = value_cache.reshape(NSLOT, NH, HD)

    zrow = np.zeros((1, HDPC), dtype=bfloat16)
    in_maps = []
    for c in range(N_CORES):
        h0 = c * HPC
        r0, r1 = h0 * HD, (h0 + HPC) * HD
        wqkT8 = np.ascontiguousarray(
            np.concatenate([wq[r0:r1], wk[r0:r1]],
                           axis=0).T * 16.0).astype(float8_e4m3)
        wvT = np.ascontiguousarray(wv[r0:r1].T).astype(bfloat16)
        woT = np.ascontiguousarray(out_w[:, r0:r1].T).astype(bfloat16)
        ks = np.concatenate(
            [np.ascontiguousarray(
                kc[:, h0:h0 + HPC, :].reshape(NSLOT, HDPC)).astype(bfloat16),
             zrow], axis=0)
        vs = np.concatenate(
            [np.ascontiguousarray(
                vc[:, h0:h0 + HPC, :].reshape(NSLOT, HDPC)).astype(bfloat16),
             zrow], axis=0)
        in_maps.append({
            "hiddenT": hiddenT, "hidden8": hidden8, "wqkT8": wqkT8,
            "wvT": wvT, "woT": woT,
            "ksrc": ks, "vsrc": vs,
            "idxs": plan['catidx'], "maskd": plan['mask'],
            "fixmd": plan['fixm'], "fixmTd": plan['fixmT'],
            "qcosd": plan['qcos'], "qsind": plan['qsin'],
            "kcosd": plan['cos_rep'], "ksind": plan['sin_rep'],
        })
    return in_maps


def kernel(hidden_states, qkv_w, out_w, cos_sin_cache, key_cache, value_cache,
           position_ids, block_tables, slots, context_lengths):
    from concourse.bass_utils import run_bass_kernel_spmd

    hidden_states = np.asarray(hidden_states, dtype=np.float32)
    qkv_w = np.asarray(qkv_w, dtype=np.float32)
    out_w = np.asarray(out_w, dtype=np.float32)
    cos_sin_cache = np.asarray(cos_sin_cache, dtype=np.float32)
    key_cache = np.asarray(key_cache, dtype=np.float32)
    value_cache = np.asarray(value_cache, dtype=np.float32)

    plan = _plan(np.asarray(position_ids), np.asarray(block_tables),
                 np.asarray(slots), np.asarray(context_lengths), cos_sin_cache)
    nc = _build_bass(plan)
    in_maps = _make_in_maps(hidden_states, qkv_w, out_w, key_cache,
                            value_cache, plan)

    res = run_bass_kernel_spmd(nc, in_maps, core_ids=list(range(N_CORES)))
    acc = np.zeros((128, 512), dtype=np.float32)
    for c in range(N_CORES):
        acc += res.results[c]["y"]
    # y[p, c*16+b] = out[b, c*128+p]
    out = acc.reshape(128, 32, B).transpose(2, 1, 0).reshape(B, HID)
    return np.ascontiguousarray(out).reshape(B, 1, HID)
